# revision 1
# baseline (speedup 1.0000x reference)
"""AFT-attention (nn_AFTAttention) distributed Bass kernel for 8 TRN2 NeuronCores.

Reference computation (B=8, T=4096, D=H=1024):
    Q = x @ Wq.T + bq ; K = x @ Wk.T + bk ; V = x @ Wv.T + bv      # [B,T,H]
    numer = exp(K + wbias)                                          # [B,T,H]
    denom = numer.sum(axis=0)                                       # [T,H]
    weighted = (numer * V).sum(axis=0) / denom                      # [T,H]
    out = sigmoid(Q) * weighted                                     # [B,T,H]

Sharding: the reductions are over the BATCH axis only, so sharding T across
the 8 cores makes every reduction core-local -- zero collectives.  Each core
handles all 8 batches for its 512-timestep slice.

Per-core layout: tiles are [h(128 partitions), t(512 free)] so every
per-feature constant (bq, bk+wbias, bv) rides the per-partition scalar port
of ScalarE activation / DVE scalar_tensor_tensor.  Host pre-transposes x to
x^T[b, d, t] and weights to W^T[d, h], and converts to bf16 (matmul runs at
1 col/cycle bf16 vs 4 cycles fp32).  sigmoid is computed via tanh (same ACT
table set as exp -> no 2.7us table reloads):
    sigmoid(q) * w == (tanh(q/2) + 1) * (w/2)

Schedule (per core, ~352us HW at 2.4GHz; PE floor for 1536 bf16 N=512
matmuls is ~332us):
 - preamble shadow: 10 dummy matmuls warm the HAM clock gate; a dummy exp
   pulls the ACT table load forward; input DMAs stream in first-consumer
   order as ~512KB batches.
 - phase A (batches 0-1): K and V GEMMs in chunk-outer waves of 8 PSUM
   banks (4 h-tiles x 2 batches) so each DMA chunk is consumed on arrival;
   K waves precede V waves to cover the trailing wv stream.
 - phases B (batches 2-7 in pairs): per-h-tile K/V with 4-bank groups;
   exp/STT/adds run on ScalarE/VectorE/GpSimd behind the PE.
 - phase C: weighted_half = 0.5*nv/ns (reciprocal on DVE), then Q GEMMs +
   (tanh+1)*wh epilogue, bf16 outputs DMA'd per h-tile; the last h-tile
   uses smaller trailing groups to shorten the kernel tail.
A post-finalize pass drops ~1000 redundant LDWEIGHTS (walrus ldw-opt is
disabled in this container).  Host assembles/transposes the bf16 [B,H,TC]
per-core outputs into the f32 [B,T,H] result.
"""

import sys

import numpy as np

for _p in ("/opt/trn_rl_repo", "/opt/pypackages"):
    if _p not in sys.path:
        sys.path.append(_p)

B, T, D, H = 8, 4096, 1024, 1024
NCORES = 8
TC = T // NCORES      # 512 timesteps per core
P = 128               # partitions
NCH = D // P          # 8 contraction chunks
NHT = H // P          # 8 h tiles
BG = 4                # batch group size for PSUM bank pressure (Q phase)
BGA = 2               # batch group size for the K/V phases

_cached = None        # (nc, run_fn)


def _build_bass():
    import concourse.bass as bass
    import concourse.mybir as mybir
    import concourse.tile as tile
    from concourse import bacc

    f32 = mybir.dt.float32
    bf16 = mybir.dt.bfloat16
    AF = mybir.ActivationFunctionType
    OP = mybir.AluOpType

    nc = bacc.Bacc(None)

    xt = nc.declare_dram_parameter("xt", [B, NCH, P, TC], bf16, isOutput=False)
    wk = nc.declare_dram_parameter("wk", [NCH, P, H], bf16, isOutput=False)
    wv = nc.declare_dram_parameter("wv", [NCH, P, H], bf16, isOutput=False)
    wq = nc.declare_dram_parameter("wq", [NCH, P, H], bf16, isOutput=False)
    # per-partition constants, host-prearranged as [P, NHT]
    bkw = nc.declare_dram_parameter("bkw", [P, NHT], f32, isOutput=False)  # bk + wbias
    bvp = nc.declare_dram_parameter("bvp", [P, NHT], f32, isOutput=False)  # bv
    bqh = nc.declare_dram_parameter("bqh", [P, NHT], f32, isOutput=False)  # 0.5*bq
    outt = nc.declare_dram_parameter("outt", [B, H, TC], bf16, isOutput=True)

    from contextlib import ExitStack

    with tile.TileContext(nc) as tc, ExitStack() as ctx:
        sing = ctx.enter_context(tc.tile_pool(name="sing", bufs=1))
        ps = ctx.enter_context(tc.tile_pool(name="ps", bufs=8, space="PSUM"))
        acc = ctx.enter_context(tc.tile_pool(name="acc", bufs=2))
        tmp = ctx.enter_context(tc.tile_pool(name="tmp", bufs=6))
        outp = ctx.enter_context(tc.tile_pool(name="outp", bufs=6))

        # --- resident inputs -------------------------------------------------
        # weights: [P, NCH, H] per matrix; lhsT slice = w_sb[:, c, j*P:(j+1)*P]
        wk_sb = sing.tile([P, NCH, H], bf16)
        wv_sb = sing.tile([P, NCH, H], bf16)
        wq_sb = sing.tile([P, NCH, H], bf16)
        # x^T: [P, B, NCH, TC]; rhs slice = xt_sb[:, b, c, :]
        xt_sb = sing.tile([P, B, NCH, TC], bf16)
        bkw_sb = sing.tile([P, NHT], f32)
        bvp_sb = sing.tile([P, NHT], f32)
        bqh_sb = sing.tile([P, NHT], f32)

        # PE warm-up: ~10 dummy matmuls on never-written SBUF scratch issue
        # immediately after the preamble (no data deps), so the HAM clock
        # gate reaches 8/8 before the first real matmul's data lands.
        warm_w = sing.tile([P, P], bf16)
        warm_x = sing.tile([P, TC], bf16)
        nc.vector.memset(warm_w, 0.0)
        nc.vector.memset(warm_x, 0.0)
        warm_ps = ps.tile([P, TC], f32, tag="ps", name="warm_ps")
        for _ in range(10):
            nc.tensor.matmul(warm_ps, warm_w, warm_x, start=True, stop=True)
        # also pull the ~2.7us ACT table load (exp/tanh set) into the DMA
        # shadow instead of paying it on the first real exp
        warm_act = sing.tile([P, 1], f32)
        nc.scalar.activation(out=warm_act, in_=warm_w[:, 0:1], func=AF.Exp)

        # batched per-chunk DMAs (one per (c, batch-group): ~512KB each),
        # emitted in first-consumer order so the K matmuls of (j=0, g=0)
        # can start as soon as wk[c]+xt[b0..3][c] land; biases are not needed
        # until the first exp (after the first K chunk loop), so they load
        # after the first chunk pair
        for c in range(NCH):
            nc.sync.dma_start(out=wk_sb[:, c, :], in_=wk[c])
            nc.sync.dma_start(
                out=xt_sb[:, 0:BGA, c, :],
                in_=xt[0:BGA, c].rearrange("b p t -> p b t"),
            )
            if c == 0:
                nc.sync.dma_start(out=bkw_sb, in_=bkw[:, :])
                nc.sync.dma_start(out=bvp_sb, in_=bvp[:, :])
                nc.sync.dma_start(out=bqh_sb, in_=bqh[:, :])
        for c in range(NCH):
            nc.sync.dma_start(out=wv_sb[:, c, :], in_=wv[c])
        for g in range(1, B // BGA):
            for c in range(NCH):
                nc.sync.dma_start(
                    out=xt_sb[:, g * BGA : (g + 1) * BGA, c, :],
                    in_=xt[g * BGA : (g + 1) * BGA, c].rearrange("b p t -> p b t"),
                )
        for c in range(NCH):
            nc.sync.dma_start(out=wq_sb[:, c, :], in_=wq[c])

        # --- main loops ------------------------------------------------------
        # Phase A: K/V for batch group 0 across all h tiles (only needs the
        # first half of xt -> overlaps the input DMA stream), phase B: batch
        # group 1 (completes the batch sums), phase C: Q + output.
        ns_sb = [acc.tile([P, TC], f32, tag="ns", bufs=NHT, name=f"ns_{j}") for j in range(NHT)]
        nv_sb = [acc.tile([P, TC], f32, tag="nv", bufs=NHT, name=f"nv_{j}") for j in range(NHT)]

        def kv_mms(j, bs, w_sb, out_tiles):
            hs = bass.ts(j, P)
            for c in range(NCH):
                for b in bs:
                    nc.tensor.matmul(
                        out_tiles[b], w_sb[:, c, hs], xt_sb[:, b, c, :],
                        start=(c == 0), stop=(c == NCH - 1),
                    )

        def kv_epilogue(j, bs, kps, vps):
            # numer = exp(k + bk + wbias); ns += numer; nv += numer*(v + bv)
            for b in bs:
                if b == 0:
                    numer = ns_sb[j]  # first batch: exp writes the accumulator
                else:
                    numer = tmp.tile([P, TC], f32, tag="numer", bufs=6,
                                     name=f"num_{j}_{b}")
                nc.scalar.activation(
                    out=numer, in_=kps[b], func=AF.Exp,
                    bias=bkw_sb[:, j : j + 1], scale=1.0,
                )
                if b == 0:
                    nvt = nv_sb[j]
                else:
                    nvt = tmp.tile([P, TC], f32, tag="nvt", bufs=3,
                                   name=f"nvt_{j}_{b}")
                nc.vector.scalar_tensor_tensor(
                    out=nvt, in0=vps[b], scalar=bvp_sb[:, j : j + 1],
                    in1=numer, op0=OP.add, op1=OP.mult,
                )
                if b != 0:
                    nc.gpsimd.tensor_add(ns_sb[j], ns_sb[j], numer)
                    nc.vector.tensor_add(nv_sb[j], nv_sb[j], nvt)

        for g in range(B // BGA):
            bs = range(g * BGA, (g + 1) * BGA)
            if g == 0:
                # Chunk-outer waves: 4 h-tiles x 2 batches = 8 PSUM banks
                # accumulate together, consuming each (wk[c], xt[c]) DMA the
                # moment it lands (no head-of-line block on a later chunk).
                # K waves run first so the trailing wv stream has time.
                numer_st = {}

                def k_wave(jlist):
                    kq = {j: {b: ps.tile([P, TC], f32, tag="ps", name=f"kps_{j}_{b}") for b in bs}
                          for j in jlist}
                    for c in range(NCH):
                        for j in jlist:
                            for b in bs:
                                nc.tensor.matmul(
                                    kq[j][b], wk_sb[:, c, bass.ts(j, P)], xt_sb[:, b, c, :],
                                    start=(c == 0), stop=(c == NCH - 1),
                                )
                    for j in jlist:
                        for b in bs:
                            if b == 0:
                                numer = ns_sb[j]
                            else:
                                numer = tmp.tile([P, TC], f32, tag="numer", bufs=6,
                                                 name=f"numw_{j}_{b}")
                            nc.scalar.activation(
                                out=numer, in_=kq[j][b], func=AF.Exp,
                                bias=bkw_sb[:, j : j + 1], scale=1.0,
                            )
                            numer_st[(j, b)] = numer

                def v_wave(jlist):
                    vq = {j: {b: ps.tile([P, TC], f32, tag="ps", name=f"vps_{j}_{b}") for b in bs}
                          for j in jlist}
                    for c in range(NCH):
                        for j in jlist:
                            for b in bs:
                                nc.tensor.matmul(
                                    vq[j][b], wv_sb[:, c, bass.ts(j, P)], xt_sb[:, b, c, :],
                                    start=(c == 0), stop=(c == NCH - 1),
                                )
                    for j in jlist:
                        for b in bs:
                            numer = numer_st.pop((j, b))
                            if b == 0:
                                nvt = nv_sb[j]
                            else:
                                nvt = tmp.tile([P, TC], f32, tag="nvt", bufs=3,
                                               name=f"nvtw_{j}_{b}")
                            nc.vector.scalar_tensor_tensor(
                                out=nvt, in0=vq[j][b], scalar=bvp_sb[:, j : j + 1],
                                in1=numer, op0=OP.add, op1=OP.mult,
                            )
                            if b != 0:
                                nc.gpsimd.tensor_add(ns_sb[j], ns_sb[j], numer)
                                nc.vector.tensor_add(nv_sb[j], nv_sb[j], nvt)

                k_wave(range(0, 4))
                v_wave(range(0, 4))
                k_wave(range(4, 8))
                v_wave(range(4, 8))
            else:
                for j in range(NHT):
                    kps = {b: ps.tile([P, TC], f32, tag="ps", name=f"kps_{j}_{b}") for b in bs}
                    vps = {b: ps.tile([P, TC], f32, tag="ps", name=f"vps_{j}_{b}") for b in bs}
                    kv_mms(j, bs, wk_sb, kps)
                    kv_mms(j, bs, wv_sb, vps)
                    kv_epilogue(j, bs, kps, vps)

        # weighted_half(j) = 0.5 * nv / ns, then phase C: Q + out
        wh = [tmp.tile([P, TC], bf16, tag="wh", bufs=NHT, name=f"wh_{j}") for j in range(NHT)]
        for j in range(NHT):
            rec = tmp.tile([P, TC], f32, tag="rec", bufs=2)
            nc.vector.reciprocal(rec, ns_sb[j])
            nc.vector.scalar_tensor_tensor(
                out=wh[j], in0=nv_sb[j], scalar=0.5, in1=rec, op0=OP.mult, op1=OP.mult,
            )

        for j in range(NHT):
            hs = bass.ts(j, P)
            # Q matmuls + out = (tanh(q/2 + bq/2) + 1) * weighted_half
            # (last h tile: smaller trailing groups shorten the kernel tail)
            groups = [range(0, 4), range(4, 8)] if j < NHT - 1 else [
                range(0, 4), range(4, 6), range(6, 8)]
            for bs in groups:
                qps = {b: ps.tile([P, TC], f32, tag="ps", name=f"qps_{j}_{b}") for b in bs}
                for c in range(NCH):
                    for b in bs:
                        nc.tensor.matmul(
                            qps[b], wq_sb[:, c, hs], xt_sb[:, b, c, :],
                            start=(c == 0), stop=(c == NCH - 1),
                        )
                for b in bs:
                    th = tmp.tile([P, TC], f32, tag="th", bufs=4)
                    nc.scalar.activation(
                        out=th, in_=qps[b], func=AF.Tanh,
                        bias=bqh_sb[:, j : j + 1], scale=0.5,
                    )
                    ot = outp.tile([P, TC], bf16, tag="ot")
                    nc.vector.scalar_tensor_tensor(
                        out=ot, in0=th, scalar=1.0, in1=wh[j], op0=OP.add, op1=OP.mult,
                    )
                    nc.sync.dma_start(out=outt[b, hs, :], in_=ot)

    nc.finalize()
    _dedup_ldweights(nc)
    return nc


def _dedup_ldweights(nc):
    """Drop InstLdweights that reload the exact weights already resident in
    the PE array (walrus's ldw-opt is disabled in this container, so every
    matmul otherwise gets its own LDWEIGHTS).  Sync carried by a removed
    LDWEIGHTS is preserved on an InstEventSemaphore in its place."""
    import concourse.mybir as mybir

    for bb in nc.m.functions[0].blocks:
        insts = list(bb.instructions)
        new = []
        prev_key = None
        changed = False
        for inst in insts:
            tname = type(inst).__name__
            if str(inst.engine) != "EngineType.PE":
                new.append(inst)
                continue
            if tname == "InstLdweights":
                key = (
                    str(inst.ins[0]),
                    str(inst.perf_mode),
                    str(inst.is_transpose),
                    str(inst.tile_position),
                )
                if key == prev_key:
                    si = inst.sync_info
                    if si is not None and (si.on_wait or si.on_update):
                        new.append(
                            mybir.InstEventSemaphore(
                                name=inst.name,
                                engine=inst.engine,
                                sync_info=si,
                                ins=[],
                                outs=[],
                            )
                        )
                    changed = True
                    continue
                prev_key = key
                new.append(inst)
            elif tname == "InstMatmult":
                new.append(inst)
            else:
                prev_key = None  # branches/drains: be conservative
                new.append(inst)
        if changed:
            del bb.instructions[:]
            for inst in new:
                bb.add_instruction(inst)


def _prepare_in_maps(x, Wq, bq, Wk, bk, Wv, bv, wbias):
    import ml_dtypes

    bf16 = ml_dtypes.bfloat16
    f32 = np.float32

    # weights: W.T [D, H] -> [NCH, P, H] bf16 (shared by all cores)
    def prep_w(w):
        return np.ascontiguousarray(w.T.astype(bf16)).reshape(NCH, P, H)

    wq_h = prep_w(np.asarray(Wq))
    wk_h = prep_w(np.asarray(Wk))
    wv_h = prep_w(np.asarray(Wv))

    # per-partition constants as [P, NHT]: col j holds values for h in [j*128, ...)
    def prep_b(v):
        return np.ascontiguousarray(np.asarray(v, f32).reshape(NHT, P).T)

    bkw_h = prep_b(np.asarray(bk, f32) + np.asarray(wbias, f32))
    bvp_h = prep_b(bv)
    bqh_h = prep_b(0.5 * np.asarray(bq, f32))

    x = np.asarray(x)
    in_maps = []
    for core in range(NCORES):
        xs = x[:, core * TC : (core + 1) * TC, :]           # [B, TC, D]
        xtc = np.ascontiguousarray(xs.transpose(0, 2, 1).astype(bf16)).reshape(
            B, NCH, P, TC
        )
        in_maps.append(
            {
                "xt": xtc,
                "wq": wq_h,
                "wk": wk_h,
                "wv": wv_h,
                "bkw": bkw_h,
                "bvp": bvp_h,
                "bqh": bqh_h,
            }
        )
    return in_maps


def _get_nc():
    global _cached
    if _cached is None:
        _cached = _build_bass()
    return _cached


TRACE = False          # set True from a test harness to profile
TRACE_TMPDIR = None    # optional persistent dir for trace artifacts
LAST_RESULT = None     # BassKernelResults of the most recent kernel() call


def kernel(x, Wq, bq, Wk, bk, Wv, bv, wbias):
    global LAST_RESULT
    from concourse.bass_utils import run_bass_kernel_spmd

    nc = _get_nc()
    in_maps = _prepare_in_maps(x, Wq, bq, Wk, bk, Wv, bv, wbias)
    kw = {}
    if TRACE:
        kw = {"trace": True, "tmpdir": TRACE_TMPDIR}
    res = run_bass_kernel_spmd(nc, in_maps, core_ids=list(range(NCORES)), **kw)
    LAST_RESULT = res
    out = np.empty((B, T, H), np.float32)
    for core in range(NCORES):
        o = np.asarray(res.results[core]["outt"])            # [B, H, TC] bf16
        out[:, core * TC : (core + 1) * TC, :] = o.astype(np.float32).transpose(
            0, 2, 1
        )
    return out



# revision 2
# speedup vs baseline: 1.2400x; 1.2400x over previous
"""AFT-attention (nn_AFTAttention) distributed Bass kernel for 8 TRN2 NeuronCores.

Reference computation (B=8, T=4096, D=H=1024):
    Q = x @ Wq.T + bq ; K = x @ Wk.T + bk ; V = x @ Wv.T + bv      # [B,T,H]
    numer = exp(K + wbias)                                          # [B,T,H]
    denom = numer.sum(axis=0)                                       # [T,H]
    weighted = (numer * V).sum(axis=0) / denom                      # [T,H]
    out = sigmoid(Q) * weighted                                     # [B,T,H]

Sharding: the reductions are over the BATCH axis only, so sharding T across
the 8 cores makes every reduction core-local -- zero collectives.  Each core
handles all 8 batches for its 512-timestep slice.

Precision strategy (rel-err budget 2e-2; measured ~1.6e-2 in emulation):
 - Q GEMM entirely in fp8 e4m3 with DoubleRow perf mode (2x PE throughput;
   4 matmuls per [128,512] tile instead of 8).  sigmoid damps the fp8 noise
   (~1% end-to-end contribution).
 - K GEMM mixed: contraction chunks 0-3 as fp8 DoubleRow pairs, chunks 4-7
   bf16 (6 matmuls instead of 8).  exp() amplifies K noise, but the batch-
   normalised weighting absorbs half-fp8 (~1.3% contribution).
 - V GEMM all bf16 (V noise passes straight through to the output).
 Scales (all powers of 2, exact): x8 = e4m3(16*x), w*8 = e4m3(256*W),
 wk16 = bf16(4096*Wk) so bf16 K chunks accumulate on the same 4096x scale
 as the fp8 pairs; the 1/4096 descale folds into the activation scale.

Per-core layout: tiles are [h(128 partitions), t(512 free)] so every
per-feature constant (bq, bk+wbias, bv) rides the per-partition scalar port
of ScalarE activation / DVE scalar_tensor_tensor.  Host pre-transposes x to
x^T[b, d, t] and weights to W^T[d, h].  sigmoid is computed via tanh (same
ACT table set as exp -> no 2.7us table reloads):
    sigmoid(q) * w == (tanh(q/2) + 1) * (w/2)

Schedule (per core; PE floor for 64x(6 K + 8 V + 4 Q) slots is ~246us):
 - preamble shadow: dummy matmuls (bf16 + one fp8 DR) warm the HAM clock
   gate; a dummy exp pulls the ACT table load forward; input DMAs stream in
   first-consumer order.
 - phase A (batches 0-1): K and V GEMMs in chunk-outer waves of 8 PSUM
   banks (4 h-tiles x 2 batches) so each DMA chunk is consumed on arrival;
   V waves run their bf16 chunks 4-7 first (already resident from K).
 - phases B (batches 2-7 in pairs): per-h-tile K/V with 4-bank groups;
   exp/STT/adds run on ScalarE/VectorE/GpSimd behind the PE.
 - phase C: weighted_half = 0.5*nv/ns (reciprocal on DVE), then Q DR GEMMs
   + (tanh+1)*wh epilogue, bf16 outputs DMA'd per h-tile; the last h-tile
   uses smaller trailing groups to shorten the kernel tail.
A post-finalize pass drops redundant LDWEIGHTS (walrus ldw-opt is disabled
in this container).  Host assembles/transposes the bf16 [B,H,TC] per-core
outputs into the f32 [B,T,H] result.
"""

import sys

import numpy as np

for _p in ("/opt/trn_rl_repo", "/opt/pypackages"):
    if _p not in sys.path:
        sys.path.append(_p)

B, T, D, H = 8, 4096, 1024, 1024
NCORES = 8
TC = T // NCORES      # 512 timesteps per core
P = 128               # partitions
NCH = D // P          # 8 contraction chunks
NK8 = 4               # K chunks 0..NK8-1 run fp8 (must be even)
NHT = H // P          # 8 h tiles
BG = 4                # batch group size for PSUM bank pressure (Q phase)
BGA = 2               # batch group size for the K/V phases

SX = 16.0             # x fp8 scale
SW = 256.0            # weight fp8 scale
SKQ = SX * SW         # 4096: accumulated scale of fp8 (and pre-scaled bf16) K/Q

_cached = None        # (nc, run_fn)


def _build_bass():
    import concourse.bass as bass
    import concourse.mybir as mybir
    import concourse.tile as tile
    from concourse import bacc

    f32 = mybir.dt.float32
    bf16 = mybir.dt.bfloat16
    f8 = mybir.dt.float8e4
    AF = mybir.ActivationFunctionType
    OP = mybir.AluOpType
    DR = mybir.MatmulPerfMode.DoubleRow

    nc = bacc.Bacc(None)

    xt = nc.declare_dram_parameter("xt", [B, NCH, P, TC], bf16, isOutput=False)
    x8 = nc.declare_dram_parameter("x8", [B, NCH, P, TC], f8, isOutput=False)
    wk8 = nc.declare_dram_parameter("wk8", [NK8, P, H], f8, isOutput=False)
    wk16 = nc.declare_dram_parameter("wk16", [NCH - NK8, P, H], bf16, isOutput=False)
    wv = nc.declare_dram_parameter("wv", [NCH, P, H], bf16, isOutput=False)
    wq8 = nc.declare_dram_parameter("wq8", [NCH, P, H], f8, isOutput=False)
    # per-partition constants, host-prearranged as [P, NHT]
    bkw = nc.declare_dram_parameter("bkw", [P, NHT], f32, isOutput=False)  # bk + wbias
    bvp = nc.declare_dram_parameter("bvp", [P, NHT], f32, isOutput=False)  # bv
    bqh = nc.declare_dram_parameter("bqh", [P, NHT], f32, isOutput=False)  # 0.5*bq
    outt = nc.declare_dram_parameter("outt", [B, H, TC], bf16, isOutput=True)

    from contextlib import ExitStack

    with tile.TileContext(nc) as tc, ExitStack() as ctx:
        sing = ctx.enter_context(tc.tile_pool(name="sing", bufs=1))
        ps = ctx.enter_context(tc.tile_pool(name="ps", bufs=8, space="PSUM"))
        acc = ctx.enter_context(tc.tile_pool(name="acc", bufs=2))
        tmp = ctx.enter_context(tc.tile_pool(name="tmp", bufs=6))
        outp = ctx.enter_context(tc.tile_pool(name="outp", bufs=4))

        # --- resident inputs -------------------------------------------------
        # weights: lhsT slice = w_sb[:, c, j*P:(j+1)*P] (fp8 DR: [:, 2c:2c+2, hs])
        wk8_sb = sing.tile([P, NK8, H], f8)
        wk16_sb = sing.tile([P, NCH - NK8, H], bf16)
        wv_sb = sing.tile([P, NCH, H], bf16)
        wq8_sb = sing.tile([P, NCH, H], f8)
        # x^T: [P, B, NCH, TC]; rhs slice = xt_sb[:, b, c, :]
        xt_sb = sing.tile([P, B, NCH, TC], bf16)
        x8_sb = sing.tile([P, B, NCH, TC], f8)
        bkw_sb = sing.tile([P, NHT], f32)
        bvp_sb = sing.tile([P, NHT], f32)
        bqh_sb = sing.tile([P, NHT], f32)

        # PE warm-up: dummy matmuls on never-written SBUF scratch issue
        # immediately after the preamble (no data deps), so the HAM clock
        # gate reaches 8/8 before the first real matmul's data lands.
        warm_w = sing.tile([P, P], bf16)
        warm_x = sing.tile([P, TC], bf16)
        warm_w8 = sing.tile([P, 2, P], f8)
        warm_x8 = sing.tile([P, 2, TC], f8)
        nc.vector.memset(warm_w, 0.0)
        nc.vector.memset(warm_x, 0.0)
        nc.vector.memset(warm_w8, 0.0)
        nc.vector.memset(warm_x8, 0.0)
        warm_ps = ps.tile([P, TC], f32, tag="ps", name="warm_ps")
        for _ in range(10):
            nc.tensor.matmul(warm_ps, warm_w, warm_x, start=True, stop=True)
        nc.tensor.matmul(warm_ps, warm_w8, warm_x8, start=True, stop=True,
                         perf_mode=DR)
        # also pull the ~2.7us ACT table load (exp/tanh set) into the DMA
        # shadow instead of paying it on the first real exp
        warm_act = sing.tile([P, 1], f32)
        nc.scalar.activation(out=warm_act, in_=warm_w[:, 0:1], func=AF.Exp)

        # batched DMAs in first-consumer order.  The phase-A K wave consumes
        # fp8 pairs (0,1),(2,3) then bf16 chunks 4-7; the V wave consumes
        # bf16 chunks 4-7 (resident by then) then 0-3.  Biases load after the
        # first chunk pair (first consumer is the first exp).
        def dma_x8(bs, cs):
            for b in bs:
                for c in cs:
                    nc.sync.dma_start(out=x8_sb[:, b, c, :], in_=x8[b, c])

        def dma_xt(bs, cs):
            for b in bs:
                for c in cs:
                    nc.sync.dma_start(out=xt_sb[:, b, c, :], in_=xt[b, c])

        g0 = range(0, BGA)
        for c in range(NK8):
            nc.sync.dma_start(out=wk8_sb[:, c, :], in_=wk8[c])
            dma_x8(g0, [c])
            if c == 0:
                nc.sync.dma_start(out=bkw_sb, in_=bkw[:, :])
                nc.sync.dma_start(out=bvp_sb, in_=bvp[:, :])
                nc.sync.dma_start(out=bqh_sb, in_=bqh[:, :])
        for c in range(NK8, NCH):
            nc.sync.dma_start(out=wk16_sb[:, c - NK8, :], in_=wk16[c - NK8])
            dma_xt(g0, [c])
        for c in range(NK8, NCH):
            nc.sync.dma_start(out=wv_sb[:, c, :], in_=wv[c])
        for c in range(NK8):
            nc.sync.dma_start(out=wv_sb[:, c, :], in_=wv[c])
            dma_xt(g0, [c])
        for g in range(1, B // BGA):
            bs = range(g * BGA, (g + 1) * BGA)
            dma_x8(bs, range(NK8))
            dma_xt(bs, range(NK8, NCH))
            dma_xt(bs, range(NK8))
        for c in range(NCH):
            nc.sync.dma_start(out=wq8_sb[:, c, :], in_=wq8[c])
        for g in range(B // BGA):
            dma_x8(range(g * BGA, (g + 1) * BGA), range(NK8, NCH))

        # --- main loops ------------------------------------------------------
        ns_sb = [acc.tile([P, TC], f32, tag="ns", bufs=NHT, name=f"ns_{j}") for j in range(NHT)]
        nv_sb = [acc.tile([P, TC], f32, tag="nv", bufs=NHT, name=f"nv_{j}") for j in range(NHT)]

        def k_mms(j, b, out_ps):
            """Mixed-precision K accumulation: fp8 DR pairs then bf16 chunks."""
            hs = bass.ts(j, P)
            for cp in range(NK8 // 2):
                nc.tensor.matmul(
                    out_ps, wk8_sb[:, 2 * cp : 2 * cp + 2, hs],
                    x8_sb[:, b, 2 * cp : 2 * cp + 2, :],
                    start=(cp == 0), stop=False, perf_mode=DR,
                )
            for c in range(NK8, NCH):
                nc.tensor.matmul(
                    out_ps, wk16_sb[:, c - NK8, hs], xt_sb[:, b, c, :],
                    start=False, stop=(c == NCH - 1),
                )

        def v_mms(j, b, out_ps):
            hs = bass.ts(j, P)
            # chunks 4-7 first (resident earliest), then 0-3
            order = list(range(NK8, NCH)) + list(range(NK8))
            for i, c in enumerate(order):
                nc.tensor.matmul(
                    out_ps, wv_sb[:, c, hs], xt_sb[:, b, c, :],
                    start=(i == 0), stop=(i == NCH - 1),
                )

        def kv_epilogue(j, bs, kps, vps):
            # numer = exp(K/4096 + bk + wbias); ns += numer; nv += numer*(v + bv)
            for b in bs:
                if b == 0:
                    numer = ns_sb[j]  # first batch: exp writes the accumulator
                else:
                    numer = tmp.tile([P, TC], f32, tag="numer", bufs=5,
                                     name=f"num_{j}_{b}")
                nc.scalar.activation(
                    out=numer, in_=kps[b], func=AF.Exp,
                    bias=bkw_sb[:, j : j + 1], scale=1.0 / SKQ,
                )
                if b == 0:
                    nvt = nv_sb[j]
                else:
                    nvt = tmp.tile([P, TC], f32, tag="nvt", bufs=3,
                                   name=f"nvt_{j}_{b}")
                nc.vector.scalar_tensor_tensor(
                    out=nvt, in0=vps[b], scalar=bvp_sb[:, j : j + 1],
                    in1=numer, op0=OP.add, op1=OP.mult,
                )
                if b != 0:
                    nc.gpsimd.tensor_add(ns_sb[j], ns_sb[j], numer)
                    nc.vector.tensor_add(nv_sb[j], nv_sb[j], nvt)

        for g in range(B // BGA):
            bs = range(g * BGA, (g + 1) * BGA)
            if g == 0:
                # Chunk-outer waves: 4 h-tiles x 2 batches = 8 PSUM banks
                # accumulate together, consuming each DMA the moment it
                # lands (no head-of-line block on a later chunk).
                numer_st = {}

                def k_wave(jlist):
                    kq = {j: {b: ps.tile([P, TC], f32, tag="ps", name=f"kps_{j}_{b}") for b in bs}
                          for j in jlist}
                    for cp in range(NK8 // 2):
                        for j in jlist:
                            for b in bs:
                                nc.tensor.matmul(
                                    kq[j][b], wk8_sb[:, 2 * cp : 2 * cp + 2, bass.ts(j, P)],
                                    x8_sb[:, b, 2 * cp : 2 * cp + 2, :],
                                    start=(cp == 0), stop=False, perf_mode=DR,
                                )
                    for c in range(NK8, NCH):
                        for j in jlist:
                            for b in bs:
                                nc.tensor.matmul(
                                    kq[j][b], wk16_sb[:, c - NK8, bass.ts(j, P)],
                                    xt_sb[:, b, c, :],
                                    start=False, stop=(c == NCH - 1),
                                )
                    for j in jlist:
                        for b in bs:
                            if b == 0:
                                numer = ns_sb[j]
                            else:
                                numer = tmp.tile([P, TC], f32, tag="numer", bufs=5,
                                                 name=f"numw_{j}_{b}")
                            nc.scalar.activation(
                                out=numer, in_=kq[j][b], func=AF.Exp,
                                bias=bkw_sb[:, j : j + 1], scale=1.0 / SKQ,
                            )
                            numer_st[(j, b)] = numer

                def v_wave(jlist):
                    vq = {j: {b: ps.tile([P, TC], f32, tag="ps", name=f"vps_{j}_{b}") for b in bs}
                          for j in jlist}
                    order = list(range(NK8, NCH)) + list(range(NK8))
                    for i, c in enumerate(order):
                        for j in jlist:
                            for b in bs:
                                nc.tensor.matmul(
                                    vq[j][b], wv_sb[:, c, bass.ts(j, P)], xt_sb[:, b, c, :],
                                    start=(i == 0), stop=(i == NCH - 1),
                                )
                    for j in jlist:
                        for b in bs:
                            numer = numer_st.pop((j, b))
                            if b == 0:
                                nvt = nv_sb[j]
                            else:
                                nvt = tmp.tile([P, TC], f32, tag="nvt", bufs=3,
                                               name=f"nvtw_{j}_{b}")
                            nc.vector.scalar_tensor_tensor(
                                out=nvt, in0=vq[j][b], scalar=bvp_sb[:, j : j + 1],
                                in1=numer, op0=OP.add, op1=OP.mult,
                            )
                            if b != 0:
                                nc.gpsimd.tensor_add(ns_sb[j], ns_sb[j], numer)
                                nc.vector.tensor_add(nv_sb[j], nv_sb[j], nvt)

                k_wave(range(0, 4))
                v_wave(range(0, 4))
                k_wave(range(4, 8))
                v_wave(range(4, 8))
            else:
                for j in range(NHT):
                    kps = {b: ps.tile([P, TC], f32, tag="ps", name=f"kps_{j}_{b}") for b in bs}
                    vps = {b: ps.tile([P, TC], f32, tag="ps", name=f"vps_{j}_{b}") for b in bs}
                    for b in bs:
                        k_mms(j, b, kps[b])
                    for b in bs:
                        v_mms(j, b, vps[b])
                    kv_epilogue(j, bs, kps, vps)

        # weighted_half(j) = 0.5 * nv / ns, then phase C: Q + out
        wh = [tmp.tile([P, TC], bf16, tag="wh", bufs=NHT, name=f"wh_{j}") for j in range(NHT)]
        for j in range(NHT):
            rec = tmp.tile([P, TC], f32, tag="rec", bufs=2)
            nc.vector.reciprocal(rec, ns_sb[j])
            nc.vector.scalar_tensor_tensor(
                out=wh[j], in0=nv_sb[j], scalar=0.5, in1=rec, op0=OP.mult, op1=OP.mult,
            )

        for j in range(NHT):
            hs = bass.ts(j, P)
            # Q DR matmuls + out = (tanh(q/2 + bq/2) + 1) * weighted_half
            # (last h tile: smaller trailing groups shorten the kernel tail)
            groups = [range(0, 4), range(4, 8)] if j < NHT - 1 else [
                range(0, 4), range(4, 6), range(6, 8)]
            for bs in groups:
                qps = {b: ps.tile([P, TC], f32, tag="ps", name=f"qps_{j}_{b}") for b in bs}
                for cp in range(NCH // 2):
                    for b in bs:
                        nc.tensor.matmul(
                            qps[b], wq8_sb[:, 2 * cp : 2 * cp + 2, hs],
                            x8_sb[:, b, 2 * cp : 2 * cp + 2, :],
                            start=(cp == 0), stop=(cp == NCH // 2 - 1),
                            perf_mode=DR,
                        )
                for b in bs:
                    th = tmp.tile([P, TC], f32, tag="th", bufs=3)
                    nc.scalar.activation(
                        out=th, in_=qps[b], func=AF.Tanh,
                        bias=bqh_sb[:, j : j + 1], scale=0.5 / SKQ,
                    )
                    ot = outp.tile([P, TC], bf16, tag="ot")
                    nc.vector.scalar_tensor_tensor(
                        out=ot, in0=th, scalar=1.0, in1=wh[j], op0=OP.add, op1=OP.mult,
                    )
                    nc.sync.dma_start(out=outt[b, hs, :], in_=ot)

    nc.finalize()
    _dedup_ldweights(nc)
    return nc


def _dedup_ldweights(nc):
    """Drop InstLdweights that reload the exact weights already resident in
    the PE array (walrus's ldw-opt is disabled in this container, so every
    matmul otherwise gets its own LDWEIGHTS).  Sync carried by a removed
    LDWEIGHTS is preserved on an InstEventSemaphore in its place."""
    import concourse.mybir as mybir

    for bb in nc.m.functions[0].blocks:
        insts = list(bb.instructions)
        new = []
        prev_key = None
        changed = False
        for inst in insts:
            tname = type(inst).__name__
            if str(inst.engine) != "EngineType.PE":
                new.append(inst)
                continue
            if tname == "InstLdweights":
                key = (
                    str(inst.ins[0]),
                    str(inst.perf_mode),
                    str(inst.is_transpose),
                    str(inst.tile_position),
                )
                if key == prev_key:
                    si = inst.sync_info
                    if si is not None and (si.on_wait or si.on_update):
                        new.append(
                            mybir.InstEventSemaphore(
                                name=inst.name,
                                engine=inst.engine,
                                sync_info=si,
                                ins=[],
                                outs=[],
                            )
                        )
                    changed = True
                    continue
                prev_key = key
                new.append(inst)
            elif tname == "InstMatmult":
                new.append(inst)
            else:
                prev_key = None  # branches/drains: be conservative
                new.append(inst)
        if changed:
            del bb.instructions[:]
            for inst in new:
                bb.add_instruction(inst)


def _prepare_in_maps(x, Wq, bq, Wk, bk, Wv, bv, wbias):
    import ml_dtypes

    bf16 = ml_dtypes.bfloat16
    f8 = ml_dtypes.float8_e4m3
    f32 = np.float32

    def to_f8(a, scale):
        return np.clip(np.asarray(a, f32) * scale, -240.0, 240.0).astype(f8)

    # weights: W.T [D, H] -> [NCH, P, H]; fp8 scaled by SW, K bf16 chunks
    # pre-scaled by SKQ so all K chunks accumulate on the same 4096x scale
    wq8_h = np.ascontiguousarray(to_f8(np.asarray(Wq).T, SW).reshape(NCH, P, H))
    wkT = np.asarray(Wk, f32).T
    wk8_h = np.ascontiguousarray(to_f8(wkT[: NK8 * P], SW).reshape(NK8, P, H))
    wk16_h = np.ascontiguousarray(
        (wkT[NK8 * P :] * SKQ).astype(bf16).reshape(NCH - NK8, P, H)
    )
    wv_h = np.ascontiguousarray(np.asarray(Wv).T.astype(bf16)).reshape(NCH, P, H)

    # per-partition constants as [P, NHT]: col j holds values for h in [j*128, ...)
    def prep_b(v):
        return np.ascontiguousarray(np.asarray(v, f32).reshape(NHT, P).T)

    bkw_h = prep_b(np.asarray(bk, f32) + np.asarray(wbias, f32))
    bvp_h = prep_b(bv)
    bqh_h = prep_b(0.5 * np.asarray(bq, f32))

    x = np.asarray(x)
    in_maps = []
    for core in range(NCORES):
        xs = x[:, core * TC : (core + 1) * TC, :]           # [B, TC, D]
        xst = np.ascontiguousarray(xs.transpose(0, 2, 1))   # [B, D, TC]
        xtc = xst.astype(bf16).reshape(B, NCH, P, TC)
        x8c = to_f8(xst, SX).reshape(B, NCH, P, TC)
        in_maps.append(
            {
                "xt": xtc,
                "x8": x8c,
                "wq8": wq8_h,
                "wk8": wk8_h,
                "wk16": wk16_h,
                "wv": wv_h,
                "bkw": bkw_h,
                "bvp": bvp_h,
                "bqh": bqh_h,
            }
        )
    return in_maps


def _get_nc():
    global _cached
    if _cached is None:
        _cached = _build_bass()
    return _cached


TRACE = False          # set True from a test harness to profile
TRACE_TMPDIR = None    # optional persistent dir for trace artifacts
LAST_RESULT = None     # BassKernelResults of the most recent kernel() call


def kernel(x, Wq, bq, Wk, bk, Wv, bv, wbias):
    global LAST_RESULT
    from concourse.bass_utils import run_bass_kernel_spmd

    nc = _get_nc()
    in_maps = _prepare_in_maps(x, Wq, bq, Wk, bk, Wv, bv, wbias)
    kw = {}
    if TRACE:
        kw = {"trace": True, "tmpdir": TRACE_TMPDIR}
    res = run_bass_kernel_spmd(nc, in_maps, core_ids=list(range(NCORES)), **kw)
    LAST_RESULT = res
    out = np.empty((B, T, H), np.float32)
    for core in range(NCORES):
        o = np.asarray(res.results[core]["outt"])            # [B, H, TC] bf16
        out[:, core * TC : (core + 1) * TC, :] = o.astype(np.float32).transpose(
            0, 2, 1
        )
    return out
